# revision 2
# baseline (speedup 1.0000x reference)
"""Trainium2 kernel v3 for nn_Block_7868380086724 (gnn_message_passing).

Deltas vs baseline (kernel.py):
 - center offset (k=K//2, identity neighbor) handled as one direct matmul per
   dest window from a resident xloc^T bf16 tile: removes N/ncores gathered
   slots per core (~25k descriptors).
 - dw slot counts rounded to 128 (was 512): ~21k fewer pad slots per core.
 - phase 2 (LN+MLP+residual) runs INLINE at each dw retire, hiding the old
   ~1.1 ms serial tail under the gather; no acc DRAM round-trip.
 - deeper gather-side buffering (pg/pidx/pseg bufs=6) to ride out the ~20 us
   early-run DMA stalls.
"""

import numpy as np
from contextlib import ExitStack

import concourse.bass as bass
import concourse.bacc as bacc
import concourse.mybir as mybir
import concourse.tile as tile
from concourse.bass_utils import run_bass_kernel_spmd
from concourse.masks import make_identity
import ml_dtypes

BF16H = ml_dtypes.bfloat16

D_WIN = 224
CALL = 1024


def build_plan(nbr, ncores, D=D_WIN, PSB=512):
    K, N = nbr.shape
    KC = K // 2  # center offset (0,0,0): nbr[KC] == arange(N)
    assert np.array_equal(nbr[KC], np.arange(N, dtype=nbr.dtype)), \
        "center offset is not identity"
    npc = N // ncores
    half = npc // 2
    ndw_h = -(-half // D)
    half_pad = ndw_h * D
    ndw = 2 * ndw_h
    TILE = 112
    ntile = D // TILE

    cores = []
    nblkh = 0
    for c in range(ncores):
        halves = []
        for h in range(2):
            i0 = c * npc + h * half
            i1 = i0 + (half if h == 0 else npc - half)
            sl = nbr[:, i0:i1]
            kk, ii = np.nonzero(sl >= 0)
            keep = kk != KC
            kk, ii = kk[keep], ii[keep]
            jj = sl[kk, ii].astype(np.int64)
            lo = int(jj.min())
            jj -= lo
            nblkh = max(nblkh, -(-(int(jj.max()) + 1) // 128))
            dw = ii // D
            order = np.lexsort((ii, kk, dw))
            halves.append(dict(kk=kk[order], ii=ii[order], jj=jj[order],
                               dw=dw[order], lo=lo))
        cores.append(halves)

    # run lengths L[h, dw, k] = max over cores
    L = np.zeros((2, ndw_h, K), dtype=np.int64)
    for c in range(ncores):
        for h in range(2):
            cc = cores[c][h]
            cnt = np.bincount(cc["dw"] * K + cc["kk"], minlength=ndw_h * K)
            L[h] = np.maximum(L[h], cnt.reshape(ndw_h, K))

    runoff = np.zeros((2, ndw_h, K), dtype=np.int64)
    dwslots = np.zeros((2, ndw_h), dtype=np.int64)
    dwbase = np.zeros((2, ndw_h), dtype=np.int64)
    pos = 0
    halfbase = np.zeros(3, dtype=np.int64)
    for h in range(2):
        halfbase[h] = pos
        for dw in range(ndw_h):
            off = np.cumsum(np.concatenate([[0], L[h, dw]]))
            runoff[h, dw] = off[:-1]
            tot = -(-int(off[-1]) // 128) * 128  # 128-granular (chunk) rounding
            dwslots[h, dw] = tot
            dwbase[h, dw] = pos
            pos += tot
        pos = -(-pos // CALL) * CALL  # half ends at call boundary
    halfbase[2] = pos
    nslots = pos
    nchunk = nslots // 128
    ncall = nslots // CALL

    # per-core slot assignment + gather idx + dest ids
    percore = []
    for c in range(ncores):
        gidx = np.zeros(nslots, dtype=np.int64)  # pool row per slot (0 pad)
        dest = np.full(nslots, -1.0, dtype=np.float32)  # local dest in dw
        for h in range(2):
            cc = cores[c][h]
            rid = cc["dw"] * K + cc["kk"]
            first = np.concatenate([[True], rid[1:] != rid[:-1]])
            idx = np.arange(rid.size)
            start = np.maximum.accumulate(np.where(first, idx, 0))
            rank = idx - start
            slot = dwbase[h, cc["dw"]] + runoff[h, cc["dw"], cc["kk"]] + rank
            gidx[slot] = cc["jj"]  # per-half pool row (int16-safe)
            dest[slot] = (cc["ii"] % D).astype(np.float32)
        percore.append(dict(gidx=gidx, dest=dest,
                            lo=[cores[c][0]["lo"], cores[c][1]["lo"]]))

    # conv matmul list: (a, w, k) split at PSB lines
    s2 = []
    for h in range(2):
        for dw in range(ndw_h):
            for k in range(K):
                if L[h, dw, k] == 0:
                    continue
                a = dwbase[h, dw] + runoff[h, dw, k]
                e = a + L[h, dw, k]
                while a < e:
                    stop = min(e, (a // PSB + 1) * PSB)
                    s2.append((a, stop - a, k))
                    a = stop
    s2 = np.array(s2, dtype=np.int64)

    # dw index per chunk (dwslots % 128 == 0 so chunks never straddle dws)
    chunk_dw = np.full(nchunk, -1, dtype=np.int64)  # global dw id; -1 = pad
    for h in range(2):
        for dw in range(ndw_h):
            a = dwbase[h, dw] // 128
            e = a + dwslots[h, dw] // 128
            chunk_dw[a:e] = h * ndw_h + dw

    plan = dict(
        K=K, KC=KC, N=N, ncores=ncores, npc=npc, half=half, half_pad=half_pad,
        D=D, TILE=TILE, PSB=PSB, ndw_h=ndw_h, ndw=ndw, ntile=ntile,
        nblkh=nblkh, nslots=nslots, nchunk=nchunk,
        ncall=ncall, L=L, runoff=runoff, dwslots=dwslots, dwbase=dwbase,
        halfbase=halfbase, s2=s2, chunk_dw=chunk_dw,
    )
    assert nblkh * 128 <= 32768, f"pool half too big for int16: {nblkh*128}"
    return plan, percore


def wrap_idx_calls(gidx, ncall):
    out = np.zeros((ncall, 128, CALL // 16), dtype=np.int16)
    for n in range(ncall):
        v = gidx[n * CALL : (n + 1) * CALL]
        arr = v.reshape(CALL // 16, 16).T.astype(np.int16)
        out[n] = np.tile(arr, (8, 1))
    return out


def build_pool(xF, lo0, lo1, nblkh):
    N = xF.shape[0]
    out = np.zeros((2, nblkh * 128, 128), dtype=BF16H)
    for h, lo in enumerate((lo0, lo1)):
        n = min(N - lo, nblkh * 128)
        out[h, :n, 0:96] = xF[lo : lo + n]
    return out


F32 = mybir.dt.float32
BF16 = mybir.dt.bfloat16
I16 = mybir.dt.int16
P = 128


def build_nc(plan):
    K, KC = plan["K"], plan["KC"]
    D, TILE, PSB = plan["D"], plan["TILE"], plan["PSB"]
    ndw_h, ntile = plan["ndw_h"], plan["ntile"]
    nblkh = plan["nblkh"]
    nslots, ncall, nchunk = plan["nslots"], plan["ncall"], plan["nchunk"]
    halfbase = plan["halfbase"]
    s2, chunk_dw = plan["s2"], plan["chunk_dw"]
    ndw = plan["ndw"]
    half_pad = plan["half_pad"]
    c96 = 96
    cpc = CALL // 128

    nc = bacc.Bacc(None)

    pool_d = nc.dram_tensor("pool", [2, nblkh * 128, 128], BF16, kind="ExternalInput")
    gidx_d = nc.dram_tensor("gidx", [ncall, 128, CALL // 16], I16, kind="ExternalInput")
    dest_d = nc.dram_tensor("dest", [ncall, 128, cpc], F32, kind="ExternalInput")
    iota_d = nc.dram_tensor("iotad", [1, D], BF16, kind="ExternalInput")
    wc_d = nc.dram_tensor("wc", [K, c96, c96], BF16, kind="ExternalInput")
    xloc_d = nc.dram_tensor("xloc", [2 * half_pad, c96], F32, kind="ExternalInput")
    xlt_d = nc.dram_tensor("xlt", [c96, 2 * half_pad], BF16, kind="ExternalInput")
    w1_d = nc.dram_tensor("w1", [c96, 4 * c96], F32, kind="ExternalInput")
    w2_d = nc.dram_tensor("w2", [4 * c96, c96], F32, kind="ExternalInput")
    lnvec_d = nc.dram_tensor("lnvec", [3, c96], F32, kind="ExternalInput")
    out_d = nc.dram_tensor("out", [2 * half_pad, c96], F32, kind="ExternalOutput")

    nch = 4 * c96 // P  # 3

    with ExitStack() as ctx:
        tc = ctx.enter_context(tile.TileContext(nc))
        const = ctx.enter_context(tc.tile_pool(name="const", bufs=1))

        identb = const.tile([P, P], BF16, tag="identb")
        make_identity(nc, identb[:])
        identf = const.tile([P, P], F32, tag="identf")
        make_identity(nc, identf[:])
        iota128 = const.tile([P, D], BF16, tag="iota")
        nc.sync.dma_start(out=iota128[:], in_=iota_d[0:1, :].to_broadcast([P, D]))

        # all conv weights resident: [96, K, 96] bf16
        wct = const.tile([c96, K, c96], BF16, tag="wct")
        nc.sync.dma_start(out=wct[:], in_=wc_d.rearrange("k a b -> a k b"))
        # resident x^T for the center-offset matmul: [96, ndw*D] bf16
        xlt = const.tile([c96, 2 * half_pad], BF16, tag="xlt")
        nc.sync.dma_start(out=xlt[:], in_=xlt_d[:, :])

        # phase-2 constants
        w1t = const.tile([c96, nch, P], F32, tag="w1t")
        nc.sync.dma_start(out=w1t[:], in_=w1_d.rearrange("c (h p) -> c h p", p=P))
        w2t = const.tile([P, nch, c96], F32, tag="w2t")
        nc.sync.dma_start(out=w2t[:], in_=w2_d.rearrange("(h p) c -> p h c", p=P))
        lnw_t = const.tile([P, c96], F32, tag="lnw")
        nc.sync.dma_start(out=lnw_t[:], in_=lnvec_d[0:1, :].to_broadcast([P, c96]))
        lnb_t = const.tile([P, c96], F32, tag="lnb")
        nc.sync.dma_start(out=lnb_t[:], in_=lnvec_d[1:2, :].to_broadcast([P, c96]))
        gam_t = const.tile([P, c96], F32, tag="gam")
        nc.sync.dma_start(out=gam_t[:], in_=lnvec_d[2:3, :].to_broadcast([P, c96]))
        eps_t = const.tile([P, 1], F32, tag="eps")
        nc.vector.memset(eps_t[:], 1e-6)

        with ExitStack() as p1:
            pg = p1.enter_context(tc.tile_pool(name="pg", bufs=6))
            pidx = p1.enter_context(tc.tile_pool(name="pidx", bufs=6))
            pgt = p1.enter_context(tc.tile_pool(name="pgt", bufs=3))
            pyt = p1.enter_context(tc.tile_pool(name="pyt", bufs=3))
            pyb = p1.enter_context(tc.tile_pool(name="pyb", bufs=3))
            pseg = p1.enter_context(tc.tile_pool(name="pseg", bufs=6))
            pacc = p1.enter_context(tc.tile_pool(name="pacc", bufs=2))
            p2 = p1.enter_context(tc.tile_pool(name="p2", bufs=3))
            p2s = p1.enter_context(tc.tile_pool(name="p2s", bufs=4))
            ps_g = p1.enter_context(tc.tile_pool(name="ps_g", bufs=2, space="PSUM"))
            ps_y = p1.enter_context(tc.tile_pool(name="ps_y", bufs=2, space="PSUM"))
            ps_t = p1.enter_context(tc.tile_pool(name="ps_t", bufs=2, space="PSUM"))
            ps_a = p1.enter_context(tc.tile_pool(name="ps_a", bufs=1, space="PSUM"))
            ps_p2 = p1.enter_context(tc.tile_pool(name="ps_p2", bufs=1, space="PSUM"))

            s2_by_block = {}
            for a, w, k in s2:
                s2_by_block.setdefault(int(a) // PSB, []).append((int(a), int(w), int(k)))

            state = dict(acc_ps=None, acc_ps_dw=-1, first_of_dw=False)

            def phase2_dw(pdw, accs):
                """LN + MLP + layerscale + residual for one dest window,
                reading accs [96, D] f32 (SBUF), writing out rows."""
                for t in range(ntile):
                    r0 = pdw * D + t * TILE
                    xp = ps_p2.tile([P, nch, TILE], F32, tag="p2p")
                    nc.tensor.transpose(
                        out=xp[0:TILE, 0, 0:c96], in_=accs[:, t * TILE : (t + 1) * TILE],
                        identity=identf[0:c96, 0:c96],
                    )
                    x = p2.tile([TILE, c96], F32, tag="x")
                    nc.vector.tensor_copy(out=x[:], in_=xp[0:TILE, 0, 0:c96])
                    stats = p2s.tile([TILE, 6], F32, tag="st")
                    nc.vector.bn_stats(out=stats[:], in_=x[:])
                    mv = p2s.tile([TILE, 2], F32, tag="mv")
                    nc.vector.bn_aggr(out=mv[:], in_=stats[:])
                    rstd = p2s.tile([TILE, 1], F32, tag="rs")
                    nc.scalar.activation(
                        out=rstd[:], in_=mv[:, 1:2],
                        func=mybir.ActivationFunctionType.Sqrt,
                        bias=eps_t[0:TILE, :], scale=1.0,
                    )
                    nc.vector.reciprocal(out=rstd[:], in_=rstd[:])
                    xn = p2.tile([TILE, c96], F32, tag="xn")
                    nc.vector.tensor_scalar(
                        out=xn[:], in0=x[:],
                        scalar1=mv[:, 0:1], scalar2=rstd[:],
                        op0=mybir.AluOpType.subtract,
                        op1=mybir.AluOpType.mult,
                    )
                    nc.vector.tensor_mul(out=xn[:], in0=xn[:], in1=lnw_t[0:TILE, :])
                    nc.vector.tensor_add(out=xn[:], in0=xn[:], in1=lnb_t[0:TILE, :])
                    xtp = ps_p2.tile([P, nch, TILE], F32, tag="p2p")
                    nc.tensor.transpose(
                        out=xtp[0:c96, 0, 0:TILE], in_=xn[:], identity=identf[0:TILE, 0:TILE]
                    )
                    xnt = p2.tile([c96, TILE], F32, tag="xnt")
                    nc.vector.tensor_copy(out=xnt[:], in_=xtp[0:c96, 0, 0:TILE])
                    htp = ps_p2.tile([P, nch, TILE], F32, tag="p2p")
                    for cc in range(nch):
                        nc.tensor.matmul(
                            out=htp[:, cc, :], lhsT=w1t[:, cc, :], rhs=xnt[:],
                            start=True, stop=True,
                        )
                    ht = p2.tile([P, nch, TILE], F32, tag="ht")
                    nc.scalar.activation(
                        out=ht[:], in_=htp[:],
                        func=mybir.ActivationFunctionType.Gelu,
                    )
                    ypt = ps_p2.tile([P, nch, TILE], F32, tag="p2p")
                    yp = ypt[0:TILE, 0, 0:c96]
                    for cc in range(nch):
                        nc.tensor.matmul(
                            out=yp, lhsT=ht[:, cc, :], rhs=w2t[:, cc, :],
                            start=(cc == 0), stop=(cc == nch - 1),
                        )
                    xr = p2.tile([TILE, c96], F32, tag="xr")
                    nc.sync.dma_start(out=xr[:], in_=xloc_d[r0 : r0 + TILE, :])
                    o = p2.tile([TILE, c96], F32, tag="o")
                    nc.vector.tensor_mul(out=o[:], in0=yp, in1=gam_t[0:TILE, :])
                    nc.vector.tensor_add(out=o[:], in0=o[:], in1=xr[:])
                    nc.sync.dma_start(out=out_d[r0 : r0 + TILE, :], in_=o[:])

            def retire_dw():
                pdw = state["acc_ps_dw"]
                accs = pacc.tile([c96, D], F32, tag="accs")
                nc.vector.tensor_copy(out=accs[:], in_=state["acc_ps"][:])
                phase2_dw(pdw, accs)

            for call in range(ncall):
                h = 0 if call * CALL < halfbase[1] else 1
                idxt = pidx.tile([128, CALL // 16], I16, tag="idx")
                nc.sync.dma_start(out=idxt[:], in_=gidx_d[call])
                didc = pseg.tile([128, cpc], F32, tag="didc")
                nc.sync.dma_start(out=didc[:], in_=dest_d[call])
                gt = pg.tile([128, cpc, 128], BF16, tag="g")
                nc.gpsimd.dma_gather(
                    gt[:], pool_d[h], idxt[:], CALL, CALL, 128
                )
                for blk2 in range(CALL // PSB):
                    pb = call * (CALL // PSB) + blk2
                    if pb * PSB >= nslots:
                        break
                    if all(int(chunk_dw[pb * 4 + c]) < 0 for c in range(4)):
                        continue
                    # --- transpose 4 chunks -> G^T psum bf16 [96, 512] ---
                    gtp = ps_g.tile([c96, PSB], BF16, tag="gtp")
                    for cch in range(4):
                        nc.tensor.transpose(
                            out=gtp[:, cch * 128 : (cch + 1) * 128],
                            in_=gt[:, blk2 * 4 + cch, 0:c96],
                            identity=identb[:],
                        )
                    gts = pgt.tile([c96, PSB], BF16, tag="gts")
                    nc.scalar.copy(out=gts[:], in_=gtp[:])
                    # --- conv runs of this block -> Y^T psum f32 ---
                    ytp = ps_y.tile([c96, PSB], F32, tag="ytp")
                    runs = s2_by_block.get(pb, [])
                    for ri, (a, w, k) in enumerate(runs):
                        nc.tensor.matmul(
                            out=ytp[:, a - pb * PSB : a - pb * PSB + w],
                            lhsT=wct[:, k, :],
                            rhs=gts[:, a - pb * PSB : a - pb * PSB + w],
                            start=(ri == 0),
                            stop=(ri == len(runs) - 1),
                        )
                    # zero exactly the uncovered column intervals (runs may
                    # leave mid-block gaps at 128-granular dw boundaries)
                    pos0 = 0
                    for a, w, _k in sorted(runs):
                        ra = a - pb * PSB
                        if ra > pos0:
                            nc.vector.memset(ytp[:, pos0:ra], 0.0)
                        pos0 = ra + w
                    if pos0 < PSB:
                        nc.vector.memset(ytp[:, pos0:PSB], 0.0)
                    yts = pyt.tile([c96, PSB], BF16, tag="yts")
                    nc.scalar.copy(out=yts[:], in_=ytp[:])
                    # --- transpose back -> Y psum bf16 [128, 4, 96] ---
                    ybp = ps_t.tile([128, 4, c96], BF16, tag="ybp")
                    for cch in range(4):
                        nc.tensor.transpose(
                            out=ybp[:, cch, :],
                            in_=yts[:, cch * 128 : (cch + 1) * 128],
                            identity=identb[0:c96, 0:c96],
                        )
                    ybs = pyb.tile([128, 4, c96], BF16, tag="ybs")
                    nc.scalar.copy(out=ybs[:], in_=ybp[:])
                    # --- SEG matmuls into acc^T psum ---
                    for cch in range(4):
                        ch = pb * 4 + cch
                        dwg = int(chunk_dw[ch])
                        if dwg < 0:
                            continue
                        if state["acc_ps_dw"] != dwg:
                            if state["acc_ps"] is not None:
                                retire_dw()
                            acc_new = ps_a.tile([c96, D], F32, tag="accps")
                            state["acc_ps"] = acc_new
                            state["acc_ps_dw"] = dwg
                            # open the accumulation with the center-offset
                            # contribution: acc^T += W_c^T @ x^T[:, window]
                            nc.tensor.matmul(
                                out=state["acc_ps"][:],
                                lhsT=wct[:, KC, :],
                                rhs=xlt[:, dwg * D : (dwg + 1) * D],
                                start=True, stop=False,
                            )
                        # build SEG [128, D] bf16 on DVE
                        segt = pseg.tile([128, D], BF16, tag="seg")
                        nc.vector.tensor_scalar(
                            out=segt[:],
                            in0=iota128[:],
                            scalar1=didc[:, blk2 * 4 + cch : blk2 * 4 + cch + 1],
                            scalar2=None,
                            op0=mybir.AluOpType.is_equal,
                        )
                        last = (ch + 1 == nchunk) or (int(chunk_dw[ch + 1]) != dwg)
                        nc.tensor.matmul(
                            out=state["acc_ps"][:],
                            lhsT=ybs[:, cch, :],
                            rhs=segt[:],
                            start=False,
                            stop=last,
                        )
            # retire last dw
            if state["acc_ps"] is not None:
                retire_dw()

    nc.compile()
    return nc


def make_inputs(xF, W_conv, ln_w, ln_b, W1, W2, gamma, nbr_idx, n_cores):
    plan, percore = build_plan(nbr_idx, n_cores)
    nc = build_nc(plan)
    npc, half, half_pad = plan["npc"], plan["half"], plan["half_pad"]
    wc = np.ascontiguousarray(W_conv.astype(ml_dtypes.bfloat16))
    lnvec = np.stack([ln_w, ln_b, gamma]).astype(np.float32)
    cpc = CALL // 128
    in_maps = []
    for c in range(n_cores):
        pc = percore[c]
        pool = build_pool(xF, pc["lo"][0], pc["lo"][1], plan["nblkh"])
        gidx = wrap_idx_calls(pc["gidx"], plan["ncall"])
        dest = pc["dest"].reshape(plan["ncall"], cpc, 128).transpose(0, 2, 1).copy()
        xloc = np.zeros((2 * half_pad, 96), dtype=np.float32)
        for h in range(2):
            i0 = c * npc + h * half
            n = half if h == 0 else npc - half
            xloc[h * half_pad : h * half_pad + n] = xF[i0 : i0 + n]
        in_maps.append({
            "pool": pool, "gidx": gidx, "dest": dest, "wc": wc,
            "iotad": np.arange(plan["D"], dtype=ml_dtypes.bfloat16).reshape(1, -1),
            "xloc": xloc,
            "xlt": np.ascontiguousarray(xloc.T.astype(ml_dtypes.bfloat16)),
            "w1": np.ascontiguousarray(W1, np.float32),
            "w2": np.ascontiguousarray(W2, np.float32),
            "lnvec": lnvec,
        })
    return nc, in_maps, plan


def kernel(xF, W_conv, ln_w, ln_b, W1, W2, gamma, nbr_idx, _profile=False):
    xF = np.asarray(xF, np.float32)
    nbr_idx = np.asarray(nbr_idx, np.int32)
    n_cores = 8
    nc, in_maps, plan = make_inputs(
        xF, np.asarray(W_conv, np.float32), np.asarray(ln_w, np.float32),
        np.asarray(ln_b, np.float32), np.asarray(W1, np.float32),
        np.asarray(W2, np.float32), np.asarray(gamma, np.float32),
        nbr_idx, n_cores,
    )
    res = run_bass_kernel_spmd(nc, in_maps, core_ids=list(range(n_cores)),
                               trace=_profile)
    npc, half, half_pad = plan["npc"], plan["half"], plan["half_pad"]
    outs = []
    for c in range(n_cores):
        o = res.results[c]["out"]
        outs.append(o[0:half])
        outs.append(o[half_pad : half_pad + (npc - half)])
    full = np.concatenate(outs, 0).astype(np.float32)
    if _profile:
        kernel.last_results = res
    return full


# revision 3
# speedup vs baseline: 1.0058x; 1.0058x over previous
"""Trainium2 kernel v3 for nn_Block_7868380086724 (gnn_message_passing).

Deltas vs baseline (kernel.py):
 - center offset (k=K//2, identity neighbor) handled as one direct matmul per
   dest window from a resident xloc^T bf16 tile: removes N/ncores gathered
   slots per core (~25k descriptors).
 - dw slot counts rounded to 128 (was 512): ~21k fewer pad slots per core.
 - phase 2 (LN+MLP+residual) runs INLINE at each dw retire, hiding the old
   ~1.1 ms serial tail under the gather; no acc DRAM round-trip.
 - deeper gather-side buffering (pg/pidx/pseg bufs=6) to ride out the ~20 us
   early-run DMA stalls.
"""

import numpy as np
from contextlib import ExitStack

import concourse.bass as bass
import concourse.bacc as bacc
import concourse.mybir as mybir
import concourse.tile as tile
from concourse.bass_utils import run_bass_kernel_spmd
from concourse.masks import make_identity
import ml_dtypes

BF16H = ml_dtypes.bfloat16

D_WIN = 224
CALL = 1024


def build_plan(nbr, ncores, D=D_WIN, PSB=512):
    K, N = nbr.shape
    KC = K // 2  # center offset (0,0,0): nbr[KC] == arange(N)
    assert np.array_equal(nbr[KC], np.arange(N, dtype=nbr.dtype)), \
        "center offset is not identity"
    npc = N // ncores
    half = npc // 2
    ndw_h = -(-half // D)
    half_pad = ndw_h * D
    ndw = 2 * ndw_h
    TILE = 112
    ntile = D // TILE

    cores = []
    nblkh = 0
    for c in range(ncores):
        halves = []
        for h in range(2):
            i0 = c * npc + h * half
            i1 = i0 + (half if h == 0 else npc - half)
            sl = nbr[:, i0:i1]
            kk, ii = np.nonzero(sl >= 0)
            keep = kk != KC
            kk, ii = kk[keep], ii[keep]
            jj = sl[kk, ii].astype(np.int64)
            lo = int(jj.min())
            jj -= lo
            nblkh = max(nblkh, -(-(int(jj.max()) + 1) // 128))
            dw = ii // D
            order = np.lexsort((ii, kk, dw))
            halves.append(dict(kk=kk[order], ii=ii[order], jj=jj[order],
                               dw=dw[order], lo=lo))
        cores.append(halves)

    # run lengths L[h, dw, k] = max over cores
    L = np.zeros((2, ndw_h, K), dtype=np.int64)
    for c in range(ncores):
        for h in range(2):
            cc = cores[c][h]
            cnt = np.bincount(cc["dw"] * K + cc["kk"], minlength=ndw_h * K)
            L[h] = np.maximum(L[h], cnt.reshape(ndw_h, K))

    runoff = np.zeros((2, ndw_h, K), dtype=np.int64)
    dwslots = np.zeros((2, ndw_h), dtype=np.int64)
    dwbase = np.zeros((2, ndw_h), dtype=np.int64)
    pos = 0
    halfbase = np.zeros(3, dtype=np.int64)
    for h in range(2):
        halfbase[h] = pos
        for dw in range(ndw_h):
            off = np.cumsum(np.concatenate([[0], L[h, dw]]))
            runoff[h, dw] = off[:-1]
            tot = -(-int(off[-1]) // 128) * 128  # 128-granular (chunk) rounding
            dwslots[h, dw] = tot
            dwbase[h, dw] = pos
            pos += tot
        pos = -(-pos // CALL) * CALL  # half ends at call boundary
    halfbase[2] = pos
    nslots = pos
    nchunk = nslots // 128
    ncall = nslots // CALL

    # per-core slot assignment + gather idx + dest ids
    percore = []
    for c in range(ncores):
        gidx = np.zeros(nslots, dtype=np.int64)  # pool row per slot (0 pad)
        dest = np.full(nslots, -1.0, dtype=np.float32)  # local dest in dw
        for h in range(2):
            cc = cores[c][h]
            rid = cc["dw"] * K + cc["kk"]
            first = np.concatenate([[True], rid[1:] != rid[:-1]])
            idx = np.arange(rid.size)
            start = np.maximum.accumulate(np.where(first, idx, 0))
            rank = idx - start
            slot = dwbase[h, cc["dw"]] + runoff[h, cc["dw"], cc["kk"]] + rank
            gidx[slot] = cc["jj"]  # per-half pool row (int16-safe)
            dest[slot] = (cc["ii"] % D).astype(np.float32)
        percore.append(dict(gidx=gidx, dest=dest,
                            lo=[cores[c][0]["lo"], cores[c][1]["lo"]]))

    # conv matmul list: (a, w, k) split at PSB lines
    s2 = []
    for h in range(2):
        for dw in range(ndw_h):
            for k in range(K):
                if L[h, dw, k] == 0:
                    continue
                a = dwbase[h, dw] + runoff[h, dw, k]
                e = a + L[h, dw, k]
                while a < e:
                    stop = min(e, (a // PSB + 1) * PSB)
                    s2.append((a, stop - a, k))
                    a = stop
    s2 = np.array(s2, dtype=np.int64)

    # dw index per chunk (dwslots % 128 == 0 so chunks never straddle dws)
    chunk_dw = np.full(nchunk, -1, dtype=np.int64)  # global dw id; -1 = pad
    for h in range(2):
        for dw in range(ndw_h):
            a = dwbase[h, dw] // 128
            e = a + dwslots[h, dw] // 128
            chunk_dw[a:e] = h * ndw_h + dw

    plan = dict(
        K=K, KC=KC, N=N, ncores=ncores, npc=npc, half=half, half_pad=half_pad,
        D=D, TILE=TILE, PSB=PSB, ndw_h=ndw_h, ndw=ndw, ntile=ntile,
        nblkh=nblkh, nslots=nslots, nchunk=nchunk,
        ncall=ncall, L=L, runoff=runoff, dwslots=dwslots, dwbase=dwbase,
        halfbase=halfbase, s2=s2, chunk_dw=chunk_dw,
    )
    assert nblkh * 128 <= 32768, f"pool half too big for int16: {nblkh*128}"
    return plan, percore


def wrap_idx_calls(gidx, ncall):
    out = np.zeros((ncall, 128, CALL // 16), dtype=np.int16)
    for n in range(ncall):
        v = gidx[n * CALL : (n + 1) * CALL]
        arr = v.reshape(CALL // 16, 16).T.astype(np.int16)
        out[n] = np.tile(arr, (8, 1))
    return out


def build_pool(xF, lo0, lo1, nblkh):
    N = xF.shape[0]
    out = np.zeros((2, nblkh * 128, 128), dtype=BF16H)
    for h, lo in enumerate((lo0, lo1)):
        n = min(N - lo, nblkh * 128)
        out[h, :n, 0:96] = xF[lo : lo + n]
    return out


F32 = mybir.dt.float32
BF16 = mybir.dt.bfloat16
I16 = mybir.dt.int16
P = 128


def build_nc(plan):
    K, KC = plan["K"], plan["KC"]
    D, TILE, PSB = plan["D"], plan["TILE"], plan["PSB"]
    ndw_h, ntile = plan["ndw_h"], plan["ntile"]
    nblkh = plan["nblkh"]
    nslots, ncall, nchunk = plan["nslots"], plan["ncall"], plan["nchunk"]
    halfbase = plan["halfbase"]
    s2, chunk_dw = plan["s2"], plan["chunk_dw"]
    ndw = plan["ndw"]
    half_pad = plan["half_pad"]
    c96 = 96
    cpc = CALL // 128

    nc = bacc.Bacc(None)

    pool_d = nc.dram_tensor("pool", [2, nblkh * 128, 128], BF16, kind="ExternalInput")
    gidx_d = nc.dram_tensor("gidx", [ncall, 128, CALL // 16], I16, kind="ExternalInput")
    dest_d = nc.dram_tensor("dest", [ncall, 128, cpc], F32, kind="ExternalInput")
    iota_d = nc.dram_tensor("iotad", [1, D], BF16, kind="ExternalInput")
    wc_d = nc.dram_tensor("wc", [K, c96, c96], BF16, kind="ExternalInput")
    xloc_d = nc.dram_tensor("xloc", [2 * half_pad, c96], F32, kind="ExternalInput")
    xlt_d = nc.dram_tensor("xlt", [c96, 2 * half_pad], BF16, kind="ExternalInput")
    w1_d = nc.dram_tensor("w1", [c96, 4 * c96], F32, kind="ExternalInput")
    w2_d = nc.dram_tensor("w2", [4 * c96, c96], F32, kind="ExternalInput")
    lnvec_d = nc.dram_tensor("lnvec", [3, c96], F32, kind="ExternalInput")
    out_d = nc.dram_tensor("out", [2 * half_pad, c96], F32, kind="ExternalOutput")

    nch = 4 * c96 // P  # 3

    with ExitStack() as ctx:
        tc = ctx.enter_context(tile.TileContext(nc))
        const = ctx.enter_context(tc.tile_pool(name="const", bufs=1))

        identb = const.tile([P, P], BF16, tag="identb")
        make_identity(nc, identb[:])
        identf = const.tile([P, P], F32, tag="identf")
        make_identity(nc, identf[:])
        iota128 = const.tile([P, D], BF16, tag="iota")
        nc.sync.dma_start(out=iota128[:], in_=iota_d[0:1, :].to_broadcast([P, D]))

        # all conv weights resident: [96, K, 96] bf16
        wct = const.tile([c96, K, c96], BF16, tag="wct")
        nc.sync.dma_start(out=wct[:], in_=wc_d.rearrange("k a b -> a k b"))
        # resident x^T for the center-offset matmul: [96, ndw*D] bf16
        xlt = const.tile([c96, 2 * half_pad], BF16, tag="xlt")
        nc.sync.dma_start(out=xlt[:], in_=xlt_d[:, :])

        # phase-2 constants
        w1t = const.tile([c96, nch, P], F32, tag="w1t")
        nc.sync.dma_start(out=w1t[:], in_=w1_d.rearrange("c (h p) -> c h p", p=P))
        w2t = const.tile([P, nch, c96], F32, tag="w2t")
        nc.sync.dma_start(out=w2t[:], in_=w2_d.rearrange("(h p) c -> p h c", p=P))
        lnw_t = const.tile([P, c96], F32, tag="lnw")
        nc.sync.dma_start(out=lnw_t[:], in_=lnvec_d[0:1, :].to_broadcast([P, c96]))
        lnb_t = const.tile([P, c96], F32, tag="lnb")
        nc.sync.dma_start(out=lnb_t[:], in_=lnvec_d[1:2, :].to_broadcast([P, c96]))
        gam_t = const.tile([P, c96], F32, tag="gam")
        nc.sync.dma_start(out=gam_t[:], in_=lnvec_d[2:3, :].to_broadcast([P, c96]))
        eps_t = const.tile([P, 1], F32, tag="eps")
        nc.vector.memset(eps_t[:], 1e-6)

        with ExitStack() as p1:
            pg = p1.enter_context(tc.tile_pool(name="pg", bufs=12))
            pidx = p1.enter_context(tc.tile_pool(name="pidx", bufs=12))
            pgt = p1.enter_context(tc.tile_pool(name="pgt", bufs=3))
            pyt = p1.enter_context(tc.tile_pool(name="pyt", bufs=3))
            pyb = p1.enter_context(tc.tile_pool(name="pyb", bufs=3))
            pseg = p1.enter_context(tc.tile_pool(name="pseg", bufs=12))
            pacc = p1.enter_context(tc.tile_pool(name="pacc", bufs=4))
            p2 = p1.enter_context(tc.tile_pool(name="p2", bufs=4))
            p2s = p1.enter_context(tc.tile_pool(name="p2s", bufs=4))
            ps_g = p1.enter_context(tc.tile_pool(name="ps_g", bufs=2, space="PSUM"))
            ps_y = p1.enter_context(tc.tile_pool(name="ps_y", bufs=2, space="PSUM"))
            ps_t = p1.enter_context(tc.tile_pool(name="ps_t", bufs=2, space="PSUM"))
            ps_a = p1.enter_context(tc.tile_pool(name="ps_a", bufs=1, space="PSUM"))
            ps_p2 = p1.enter_context(tc.tile_pool(name="ps_p2", bufs=1, space="PSUM"))

            s2_by_block = {}
            for a, w, k in s2:
                s2_by_block.setdefault(int(a) // PSB, []).append((int(a), int(w), int(k)))

            state = dict(acc_ps=None, acc_ps_dw=-1, first_of_dw=False)

            def phase2_dw(pdw, accs):
                """LN + MLP + layerscale + residual for one dest window,
                reading accs [96, D] f32 (SBUF), writing out rows."""
                for t in range(ntile):
                    r0 = pdw * D + t * TILE
                    xp = ps_p2.tile([P, nch, TILE], F32, tag="p2p")
                    nc.tensor.transpose(
                        out=xp[0:TILE, 0, 0:c96], in_=accs[:, t * TILE : (t + 1) * TILE],
                        identity=identf[0:c96, 0:c96],
                    )
                    x = p2.tile([TILE, c96], F32, tag="x")
                    nc.vector.tensor_copy(out=x[:], in_=xp[0:TILE, 0, 0:c96])
                    stats = p2s.tile([TILE, 6], F32, tag="st")
                    nc.vector.bn_stats(out=stats[:], in_=x[:])
                    mv = p2s.tile([TILE, 2], F32, tag="mv")
                    nc.vector.bn_aggr(out=mv[:], in_=stats[:])
                    rstd = p2s.tile([TILE, 1], F32, tag="rs")
                    nc.scalar.activation(
                        out=rstd[:], in_=mv[:, 1:2],
                        func=mybir.ActivationFunctionType.Sqrt,
                        bias=eps_t[0:TILE, :], scale=1.0,
                    )
                    nc.vector.reciprocal(out=rstd[:], in_=rstd[:])
                    xn = p2.tile([TILE, c96], F32, tag="xn")
                    nc.vector.tensor_scalar(
                        out=xn[:], in0=x[:],
                        scalar1=mv[:, 0:1], scalar2=rstd[:],
                        op0=mybir.AluOpType.subtract,
                        op1=mybir.AluOpType.mult,
                    )
                    nc.vector.tensor_mul(out=xn[:], in0=xn[:], in1=lnw_t[0:TILE, :])
                    nc.vector.tensor_add(out=xn[:], in0=xn[:], in1=lnb_t[0:TILE, :])
                    xtp = ps_p2.tile([P, nch, TILE], F32, tag="p2p")
                    nc.tensor.transpose(
                        out=xtp[0:c96, 0, 0:TILE], in_=xn[:], identity=identf[0:TILE, 0:TILE]
                    )
                    xnt = p2.tile([c96, TILE], F32, tag="xnt")
                    nc.vector.tensor_copy(out=xnt[:], in_=xtp[0:c96, 0, 0:TILE])
                    htp = ps_p2.tile([P, nch, TILE], F32, tag="p2p")
                    for cc in range(nch):
                        nc.tensor.matmul(
                            out=htp[:, cc, :], lhsT=w1t[:, cc, :], rhs=xnt[:],
                            start=True, stop=True,
                        )
                    ht = p2.tile([P, nch, TILE], F32, tag="ht")
                    nc.scalar.activation(
                        out=ht[:], in_=htp[:],
                        func=mybir.ActivationFunctionType.Gelu,
                    )
                    ypt = ps_p2.tile([P, nch, TILE], F32, tag="p2p")
                    yp = ypt[0:TILE, 0, 0:c96]
                    for cc in range(nch):
                        nc.tensor.matmul(
                            out=yp, lhsT=ht[:, cc, :], rhs=w2t[:, cc, :],
                            start=(cc == 0), stop=(cc == nch - 1),
                        )
                    xr = p2.tile([TILE, c96], F32, tag="xr")
                    nc.sync.dma_start(out=xr[:], in_=xloc_d[r0 : r0 + TILE, :])
                    o = p2.tile([TILE, c96], F32, tag="o")
                    nc.vector.tensor_mul(out=o[:], in0=yp, in1=gam_t[0:TILE, :])
                    nc.vector.tensor_add(out=o[:], in0=o[:], in1=xr[:])
                    nc.sync.dma_start(out=out_d[r0 : r0 + TILE, :], in_=o[:])

            def retire_dw():
                pdw = state["acc_ps_dw"]
                accs = pacc.tile([c96, D], F32, tag="accs")
                nc.vector.tensor_copy(out=accs[:], in_=state["acc_ps"][:])
                phase2_dw(pdw, accs)

            for call in range(ncall):
                h = 0 if call * CALL < halfbase[1] else 1
                idxt = pidx.tile([128, CALL // 16], I16, tag="idx")
                nc.sync.dma_start(out=idxt[:], in_=gidx_d[call])
                didc = pseg.tile([128, cpc], F32, tag="didc")
                nc.sync.dma_start(out=didc[:], in_=dest_d[call])
                gt = pg.tile([128, cpc, 128], BF16, tag="g")
                nc.gpsimd.dma_gather(
                    gt[:], pool_d[h], idxt[:], CALL, CALL, 128
                )
                for blk2 in range(CALL // PSB):
                    pb = call * (CALL // PSB) + blk2
                    if pb * PSB >= nslots:
                        break
                    if all(int(chunk_dw[pb * 4 + c]) < 0 for c in range(4)):
                        continue
                    # --- transpose 4 chunks -> G^T psum bf16 [96, 512] ---
                    gtp = ps_g.tile([c96, PSB], BF16, tag="gtp")
                    for cch in range(4):
                        nc.tensor.transpose(
                            out=gtp[:, cch * 128 : (cch + 1) * 128],
                            in_=gt[:, blk2 * 4 + cch, 0:c96],
                            identity=identb[:],
                        )
                    gts = pgt.tile([c96, PSB], BF16, tag="gts")
                    nc.scalar.copy(out=gts[:], in_=gtp[:])
                    # --- conv runs of this block -> Y^T psum f32 ---
                    ytp = ps_y.tile([c96, PSB], F32, tag="ytp")
                    runs = s2_by_block.get(pb, [])
                    for ri, (a, w, k) in enumerate(runs):
                        nc.tensor.matmul(
                            out=ytp[:, a - pb * PSB : a - pb * PSB + w],
                            lhsT=wct[:, k, :],
                            rhs=gts[:, a - pb * PSB : a - pb * PSB + w],
                            start=(ri == 0),
                            stop=(ri == len(runs) - 1),
                        )
                    # zero exactly the uncovered column intervals (runs may
                    # leave mid-block gaps at 128-granular dw boundaries)
                    pos0 = 0
                    for a, w, _k in sorted(runs):
                        ra = a - pb * PSB
                        if ra > pos0:
                            nc.vector.memset(ytp[:, pos0:ra], 0.0)
                        pos0 = ra + w
                    if pos0 < PSB:
                        nc.vector.memset(ytp[:, pos0:PSB], 0.0)
                    yts = pyt.tile([c96, PSB], BF16, tag="yts")
                    nc.scalar.copy(out=yts[:], in_=ytp[:])
                    # --- transpose back -> Y psum bf16 [128, 4, 96] ---
                    ybp = ps_t.tile([128, 4, c96], BF16, tag="ybp")
                    for cch in range(4):
                        nc.tensor.transpose(
                            out=ybp[:, cch, :],
                            in_=yts[:, cch * 128 : (cch + 1) * 128],
                            identity=identb[0:c96, 0:c96],
                        )
                    ybs = pyb.tile([128, 4, c96], BF16, tag="ybs")
                    nc.scalar.copy(out=ybs[:], in_=ybp[:])
                    # --- SEG matmuls into acc^T psum ---
                    for cch in range(4):
                        ch = pb * 4 + cch
                        dwg = int(chunk_dw[ch])
                        if dwg < 0:
                            continue
                        if state["acc_ps_dw"] != dwg:
                            if state["acc_ps"] is not None:
                                retire_dw()
                            acc_new = ps_a.tile([c96, D], F32, tag="accps")
                            state["acc_ps"] = acc_new
                            state["acc_ps_dw"] = dwg
                            # open the accumulation with the center-offset
                            # contribution: acc^T += W_c^T @ x^T[:, window]
                            nc.tensor.matmul(
                                out=state["acc_ps"][:],
                                lhsT=wct[:, KC, :],
                                rhs=xlt[:, dwg * D : (dwg + 1) * D],
                                start=True, stop=False,
                            )
                        # build SEG [128, D] bf16 on DVE
                        segt = pseg.tile([128, D], BF16, tag="seg")
                        nc.vector.tensor_scalar(
                            out=segt[:],
                            in0=iota128[:],
                            scalar1=didc[:, blk2 * 4 + cch : blk2 * 4 + cch + 1],
                            scalar2=None,
                            op0=mybir.AluOpType.is_equal,
                        )
                        last = (ch + 1 == nchunk) or (int(chunk_dw[ch + 1]) != dwg)
                        nc.tensor.matmul(
                            out=state["acc_ps"][:],
                            lhsT=ybs[:, cch, :],
                            rhs=segt[:],
                            start=False,
                            stop=last,
                        )
            # retire last dw
            if state["acc_ps"] is not None:
                retire_dw()

    nc.compile()
    return nc


def make_inputs(xF, W_conv, ln_w, ln_b, W1, W2, gamma, nbr_idx, n_cores):
    plan, percore = build_plan(nbr_idx, n_cores)
    nc = build_nc(plan)
    npc, half, half_pad = plan["npc"], plan["half"], plan["half_pad"]
    wc = np.ascontiguousarray(W_conv.astype(ml_dtypes.bfloat16))
    lnvec = np.stack([ln_w, ln_b, gamma]).astype(np.float32)
    cpc = CALL // 128
    in_maps = []
    for c in range(n_cores):
        pc = percore[c]
        pool = build_pool(xF, pc["lo"][0], pc["lo"][1], plan["nblkh"])
        gidx = wrap_idx_calls(pc["gidx"], plan["ncall"])
        dest = pc["dest"].reshape(plan["ncall"], cpc, 128).transpose(0, 2, 1).copy()
        xloc = np.zeros((2 * half_pad, 96), dtype=np.float32)
        for h in range(2):
            i0 = c * npc + h * half
            n = half if h == 0 else npc - half
            xloc[h * half_pad : h * half_pad + n] = xF[i0 : i0 + n]
        in_maps.append({
            "pool": pool, "gidx": gidx, "dest": dest, "wc": wc,
            "iotad": np.arange(plan["D"], dtype=ml_dtypes.bfloat16).reshape(1, -1),
            "xloc": xloc,
            "xlt": np.ascontiguousarray(xloc.T.astype(ml_dtypes.bfloat16)),
            "w1": np.ascontiguousarray(W1, np.float32),
            "w2": np.ascontiguousarray(W2, np.float32),
            "lnvec": lnvec,
        })
    return nc, in_maps, plan


def kernel(xF, W_conv, ln_w, ln_b, W1, W2, gamma, nbr_idx, _profile=False):
    xF = np.asarray(xF, np.float32)
    nbr_idx = np.asarray(nbr_idx, np.int32)
    n_cores = 8
    nc, in_maps, plan = make_inputs(
        xF, np.asarray(W_conv, np.float32), np.asarray(ln_w, np.float32),
        np.asarray(ln_b, np.float32), np.asarray(W1, np.float32),
        np.asarray(W2, np.float32), np.asarray(gamma, np.float32),
        nbr_idx, n_cores,
    )
    res = run_bass_kernel_spmd(nc, in_maps, core_ids=list(range(n_cores)),
                               trace=_profile)
    npc, half, half_pad = plan["npc"], plan["half"], plan["half_pad"]
    outs = []
    for c in range(n_cores):
        o = res.results[c]["out"]
        outs.append(o[0:half])
        outs.append(o[half_pad : half_pad + (npc - half)])
    full = np.concatenate(outs, 0).astype(np.float32)
    if _profile:
        kernel.last_results = res
    return full


# revision 4
# speedup vs baseline: 1.0185x; 1.0127x over previous
"""Trainium2 kernel v3 for nn_Block_7868380086724 (gnn_message_passing).

Deltas vs baseline (kernel.py):
 - center offset (k=K//2, identity neighbor) handled as one direct matmul per
   dest window from a resident xloc^T bf16 tile: removes N/ncores gathered
   slots per core (~25k descriptors).
 - dw slot counts rounded to 128 (was 512): ~21k fewer pad slots per core.
 - phase 2 (LN+MLP+residual) runs INLINE at each dw retire, hiding the old
   ~1.1 ms serial tail under the gather; no acc DRAM round-trip.
 - deeper gather-side buffering (pg/pidx/pseg bufs=6) to ride out the ~20 us
   early-run DMA stalls.
"""

import numpy as np
from contextlib import ExitStack

import concourse.bass as bass
import concourse.bacc as bacc
import concourse.mybir as mybir
import concourse.tile as tile
from concourse.bass_utils import run_bass_kernel_spmd
from concourse.masks import make_identity
import ml_dtypes

BF16H = ml_dtypes.bfloat16

D_WIN = 224
CALL = 1024


def build_plan(nbr, ncores, D=D_WIN, PSB=512):
    K, N = nbr.shape
    KC = K // 2  # center offset (0,0,0): nbr[KC] == arange(N)
    assert np.array_equal(nbr[KC], np.arange(N, dtype=nbr.dtype)), \
        "center offset is not identity"
    npc = N // ncores
    half = npc // 2
    ndw_h = -(-half // D)
    half_pad = ndw_h * D
    ndw = 2 * ndw_h
    TILE = 112
    ntile = D // TILE

    cores = []
    nblkh = 0
    for c in range(ncores):
        halves = []
        for h in range(2):
            i0 = c * npc + h * half
            i1 = i0 + (half if h == 0 else npc - half)
            sl = nbr[:, i0:i1]
            kk, ii = np.nonzero(sl >= 0)
            keep = kk != KC
            kk, ii = kk[keep], ii[keep]
            jj = sl[kk, ii].astype(np.int64)
            lo = int(jj.min())
            jj -= lo
            nblkh = max(nblkh, -(-(int(jj.max()) + 1) // 128))
            dw = ii // D
            order = np.lexsort((ii, kk, dw))
            halves.append(dict(kk=kk[order], ii=ii[order], jj=jj[order],
                               dw=dw[order], lo=lo))
        cores.append(halves)

    # run lengths L[h, dw, k] = max over cores
    L = np.zeros((2, ndw_h, K), dtype=np.int64)
    for c in range(ncores):
        for h in range(2):
            cc = cores[c][h]
            cnt = np.bincount(cc["dw"] * K + cc["kk"], minlength=ndw_h * K)
            L[h] = np.maximum(L[h], cnt.reshape(ndw_h, K))

    runoff = np.zeros((2, ndw_h, K), dtype=np.int64)
    dwslots = np.zeros((2, ndw_h), dtype=np.int64)
    dwbase = np.zeros((2, ndw_h), dtype=np.int64)
    pos = 0
    halfbase = np.zeros(3, dtype=np.int64)
    for h in range(2):
        halfbase[h] = pos
        for dw in range(ndw_h):
            off = np.cumsum(np.concatenate([[0], L[h, dw]]))
            runoff[h, dw] = off[:-1]
            tot = -(-int(off[-1]) // 128) * 128  # 128-granular (chunk) rounding
            dwslots[h, dw] = tot
            dwbase[h, dw] = pos
            pos += tot
        pos = -(-pos // CALL) * CALL  # half ends at call boundary
    halfbase[2] = pos
    nslots = pos
    nchunk = nslots // 128
    ncall = nslots // CALL

    # per-core slot assignment + gather idx + dest ids
    percore = []
    for c in range(ncores):
        gidx = np.zeros(nslots, dtype=np.int64)  # pool row per slot (0 pad)
        dest = np.full(nslots, -1.0, dtype=np.float32)  # local dest in dw
        for h in range(2):
            cc = cores[c][h]
            rid = cc["dw"] * K + cc["kk"]
            first = np.concatenate([[True], rid[1:] != rid[:-1]])
            idx = np.arange(rid.size)
            start = np.maximum.accumulate(np.where(first, idx, 0))
            rank = idx - start
            slot = dwbase[h, cc["dw"]] + runoff[h, cc["dw"], cc["kk"]] + rank
            gidx[slot] = cc["jj"]  # per-half pool row (int16-safe)
            dest[slot] = (cc["ii"] % D).astype(np.float32)
        percore.append(dict(gidx=gidx, dest=dest,
                            lo=[cores[c][0]["lo"], cores[c][1]["lo"]]))

    # conv matmul list: (a, w, k) split at PSB lines
    s2 = []
    for h in range(2):
        for dw in range(ndw_h):
            for k in range(K):
                if L[h, dw, k] == 0:
                    continue
                a = dwbase[h, dw] + runoff[h, dw, k]
                e = a + L[h, dw, k]
                while a < e:
                    stop = min(e, (a // PSB + 1) * PSB)
                    s2.append((a, stop - a, k))
                    a = stop
    s2 = np.array(s2, dtype=np.int64)

    # dw index per chunk (dwslots % 128 == 0 so chunks never straddle dws)
    chunk_dw = np.full(nchunk, -1, dtype=np.int64)  # global dw id; -1 = pad
    for h in range(2):
        for dw in range(ndw_h):
            a = dwbase[h, dw] // 128
            e = a + dwslots[h, dw] // 128
            chunk_dw[a:e] = h * ndw_h + dw

    plan = dict(
        K=K, KC=KC, N=N, ncores=ncores, npc=npc, half=half, half_pad=half_pad,
        D=D, TILE=TILE, PSB=PSB, ndw_h=ndw_h, ndw=ndw, ntile=ntile,
        nblkh=nblkh, nslots=nslots, nchunk=nchunk,
        ncall=ncall, L=L, runoff=runoff, dwslots=dwslots, dwbase=dwbase,
        halfbase=halfbase, s2=s2, chunk_dw=chunk_dw,
    )
    assert nblkh * 128 <= 32768, f"pool half too big for int16: {nblkh*128}"
    return plan, percore


def wrap_idx_calls(gidx, ncall):
    out = np.zeros((ncall, 128, CALL // 16), dtype=np.int16)
    for n in range(ncall):
        v = gidx[n * CALL : (n + 1) * CALL]
        arr = v.reshape(CALL // 16, 16).T.astype(np.int16)
        out[n] = np.tile(arr, (8, 1))
    return out


def build_pool(xF, lo0, lo1, nblkh):
    N = xF.shape[0]
    out = np.zeros((2, nblkh * 128, 128), dtype=BF16H)
    for h, lo in enumerate((lo0, lo1)):
        n = min(N - lo, nblkh * 128)
        out[h, :n, 0:96] = xF[lo : lo + n]
    return out


F32 = mybir.dt.float32
BF16 = mybir.dt.bfloat16
I16 = mybir.dt.int16
P = 128


def build_nc(plan):
    K, KC = plan["K"], plan["KC"]
    D, TILE, PSB = plan["D"], plan["TILE"], plan["PSB"]
    ndw_h, ntile = plan["ndw_h"], plan["ntile"]
    nblkh = plan["nblkh"]
    nslots, ncall, nchunk = plan["nslots"], plan["ncall"], plan["nchunk"]
    halfbase = plan["halfbase"]
    s2, chunk_dw = plan["s2"], plan["chunk_dw"]
    ndw = plan["ndw"]
    half_pad = plan["half_pad"]
    c96 = 96
    cpc = CALL // 128

    nc = bacc.Bacc(None)

    pool_d = nc.dram_tensor("pool", [2, nblkh * 128, 128], BF16, kind="ExternalInput")
    gidx_d = nc.dram_tensor("gidx", [ncall, 128, CALL // 16], I16, kind="ExternalInput")
    dest_d = nc.dram_tensor("dest", [ncall, 128, cpc], F32, kind="ExternalInput")
    iota_d = nc.dram_tensor("iotad", [1, D], BF16, kind="ExternalInput")
    wc_d = nc.dram_tensor("wc", [K, c96, c96], BF16, kind="ExternalInput")
    xloc_d = nc.dram_tensor("xloc", [2 * half_pad, c96], F32, kind="ExternalInput")
    xlt_d = nc.dram_tensor("xlt", [c96, 2 * half_pad], BF16, kind="ExternalInput")
    w1_d = nc.dram_tensor("w1", [c96, 4 * c96], F32, kind="ExternalInput")
    w2_d = nc.dram_tensor("w2", [4 * c96, c96], F32, kind="ExternalInput")
    lnvec_d = nc.dram_tensor("lnvec", [3, c96], F32, kind="ExternalInput")
    out_d = nc.dram_tensor("out", [2 * half_pad, c96], F32, kind="ExternalOutput")

    nch = 4 * c96 // P  # 3

    with ExitStack() as ctx:
        tc = ctx.enter_context(tile.TileContext(nc))
        const = ctx.enter_context(tc.tile_pool(name="const", bufs=1))

        identb = const.tile([P, P], BF16, tag="identb")
        make_identity(nc, identb[:])
        identf = const.tile([P, P], F32, tag="identf")
        make_identity(nc, identf[:])
        iota128 = const.tile([P, D], BF16, tag="iota")
        nc.sync.dma_start(out=iota128[:], in_=iota_d[0:1, :].to_broadcast([P, D]))

        # all conv weights resident: [96, K, 96] bf16
        wct = const.tile([c96, K, c96], BF16, tag="wct")
        nc.sync.dma_start(out=wct[:], in_=wc_d.rearrange("k a b -> a k b"))
        # resident x^T for the center-offset matmul: [96, ndw*D] bf16
        xlt = const.tile([c96, 2 * half_pad], BF16, tag="xlt")
        nc.sync.dma_start(out=xlt[:], in_=xlt_d[:, :])

        # phase-2 constants
        w1t = const.tile([c96, nch, P], F32, tag="w1t")
        nc.sync.dma_start(out=w1t[:], in_=w1_d.rearrange("c (h p) -> c h p", p=P))
        w2t = const.tile([P, nch, c96], F32, tag="w2t")
        nc.sync.dma_start(out=w2t[:], in_=w2_d.rearrange("(h p) c -> p h c", p=P))
        lnw_t = const.tile([P, c96], F32, tag="lnw")
        nc.sync.dma_start(out=lnw_t[:], in_=lnvec_d[0:1, :].to_broadcast([P, c96]))
        lnb_t = const.tile([P, c96], F32, tag="lnb")
        nc.sync.dma_start(out=lnb_t[:], in_=lnvec_d[1:2, :].to_broadcast([P, c96]))
        gam_t = const.tile([P, c96], F32, tag="gam")
        nc.sync.dma_start(out=gam_t[:], in_=lnvec_d[2:3, :].to_broadcast([P, c96]))
        eps_t = const.tile([P, 1], F32, tag="eps")
        nc.vector.memset(eps_t[:], 1e-6)

        with ExitStack() as p1:
            pg = p1.enter_context(tc.tile_pool(name="pg", bufs=16))
            pidx = p1.enter_context(tc.tile_pool(name="pidx", bufs=16))
            pgt = p1.enter_context(tc.tile_pool(name="pgt", bufs=4))
            pyt = p1.enter_context(tc.tile_pool(name="pyt", bufs=4))
            pyb = p1.enter_context(tc.tile_pool(name="pyb", bufs=4))
            pseg = p1.enter_context(tc.tile_pool(name="pseg", bufs=16))
            pacc = p1.enter_context(tc.tile_pool(name="pacc", bufs=4))
            p2 = p1.enter_context(tc.tile_pool(name="p2", bufs=4))
            p2s = p1.enter_context(tc.tile_pool(name="p2s", bufs=4))
            ps_g = p1.enter_context(tc.tile_pool(name="ps_g", bufs=2, space="PSUM"))
            ps_y = p1.enter_context(tc.tile_pool(name="ps_y", bufs=2, space="PSUM"))
            ps_t = p1.enter_context(tc.tile_pool(name="ps_t", bufs=2, space="PSUM"))
            ps_a = p1.enter_context(tc.tile_pool(name="ps_a", bufs=1, space="PSUM"))
            ps_p2 = p1.enter_context(tc.tile_pool(name="ps_p2", bufs=1, space="PSUM"))

            s2_by_block = {}
            for a, w, k in s2:
                s2_by_block.setdefault(int(a) // PSB, []).append((int(a), int(w), int(k)))

            state = dict(acc_ps=None, acc_ps_dw=-1, first_of_dw=False)

            def phase2_dw(pdw, accs):
                """LN + MLP + layerscale + residual for one dest window,
                reading accs [96, D] f32 (SBUF), writing out rows."""
                for t in range(ntile):
                    r0 = pdw * D + t * TILE
                    xp = ps_p2.tile([P, nch, TILE], F32, tag="p2p")
                    nc.tensor.transpose(
                        out=xp[0:TILE, 0, 0:c96], in_=accs[:, t * TILE : (t + 1) * TILE],
                        identity=identf[0:c96, 0:c96],
                    )
                    x = p2.tile([TILE, c96], F32, tag="x")
                    nc.vector.tensor_copy(out=x[:], in_=xp[0:TILE, 0, 0:c96])
                    stats = p2s.tile([TILE, 6], F32, tag="st")
                    nc.vector.bn_stats(out=stats[:], in_=x[:])
                    mv = p2s.tile([TILE, 2], F32, tag="mv")
                    nc.vector.bn_aggr(out=mv[:], in_=stats[:])
                    rstd = p2s.tile([TILE, 1], F32, tag="rs")
                    nc.scalar.activation(
                        out=rstd[:], in_=mv[:, 1:2],
                        func=mybir.ActivationFunctionType.Sqrt,
                        bias=eps_t[0:TILE, :], scale=1.0,
                    )
                    nc.vector.reciprocal(out=rstd[:], in_=rstd[:])
                    xn = p2.tile([TILE, c96], F32, tag="xn")
                    nc.vector.tensor_scalar(
                        out=xn[:], in0=x[:],
                        scalar1=mv[:, 0:1], scalar2=rstd[:],
                        op0=mybir.AluOpType.subtract,
                        op1=mybir.AluOpType.mult,
                    )
                    nc.vector.tensor_mul(out=xn[:], in0=xn[:], in1=lnw_t[0:TILE, :])
                    nc.vector.tensor_add(out=xn[:], in0=xn[:], in1=lnb_t[0:TILE, :])
                    xtp = ps_p2.tile([P, nch, TILE], F32, tag="p2p")
                    nc.tensor.transpose(
                        out=xtp[0:c96, 0, 0:TILE], in_=xn[:], identity=identf[0:TILE, 0:TILE]
                    )
                    xnt = p2.tile([c96, TILE], F32, tag="xnt")
                    nc.vector.tensor_copy(out=xnt[:], in_=xtp[0:c96, 0, 0:TILE])
                    htp = ps_p2.tile([P, nch, TILE], F32, tag="p2p")
                    for cc in range(nch):
                        nc.tensor.matmul(
                            out=htp[:, cc, :], lhsT=w1t[:, cc, :], rhs=xnt[:],
                            start=True, stop=True,
                        )
                    ht = p2.tile([P, nch, TILE], F32, tag="ht")
                    nc.scalar.activation(
                        out=ht[:], in_=htp[:],
                        func=mybir.ActivationFunctionType.Gelu,
                    )
                    ypt = ps_p2.tile([P, nch, TILE], F32, tag="p2p")
                    yp = ypt[0:TILE, 0, 0:c96]
                    for cc in range(nch):
                        nc.tensor.matmul(
                            out=yp, lhsT=ht[:, cc, :], rhs=w2t[:, cc, :],
                            start=(cc == 0), stop=(cc == nch - 1),
                        )
                    xr = p2.tile([TILE, c96], F32, tag="xr")
                    nc.sync.dma_start(out=xr[:], in_=xloc_d[r0 : r0 + TILE, :])
                    o = p2.tile([TILE, c96], F32, tag="o")
                    nc.vector.tensor_mul(out=o[:], in0=yp, in1=gam_t[0:TILE, :])
                    nc.vector.tensor_add(out=o[:], in0=o[:], in1=xr[:])
                    nc.sync.dma_start(out=out_d[r0 : r0 + TILE, :], in_=o[:])

            def retire_dw():
                pdw = state["acc_ps_dw"]
                accs = pacc.tile([c96, D], F32, tag="accs")
                nc.vector.tensor_copy(out=accs[:], in_=state["acc_ps"][:])
                phase2_dw(pdw, accs)

            for call in range(ncall):
                h = 0 if call * CALL < halfbase[1] else 1
                idxt = pidx.tile([128, CALL // 16], I16, tag="idx")
                nc.sync.dma_start(out=idxt[:], in_=gidx_d[call])
                didc = pseg.tile([128, cpc], F32, tag="didc")
                nc.sync.dma_start(out=didc[:], in_=dest_d[call])
                gt = pg.tile([128, cpc, 128], BF16, tag="g")
                nc.gpsimd.dma_gather(
                    gt[:], pool_d[h], idxt[:], CALL, CALL, 128
                )
                for blk2 in range(CALL // PSB):
                    pb = call * (CALL // PSB) + blk2
                    if pb * PSB >= nslots:
                        break
                    if all(int(chunk_dw[pb * 4 + c]) < 0 for c in range(4)):
                        continue
                    # --- transpose 4 chunks -> G^T psum bf16 [96, 512] ---
                    gtp = ps_g.tile([c96, PSB], BF16, tag="gtp")
                    for cch in range(4):
                        nc.tensor.transpose(
                            out=gtp[:, cch * 128 : (cch + 1) * 128],
                            in_=gt[:, blk2 * 4 + cch, 0:c96],
                            identity=identb[:],
                        )
                    gts = pgt.tile([c96, PSB], BF16, tag="gts")
                    nc.scalar.copy(out=gts[:], in_=gtp[:])
                    # --- conv runs of this block -> Y^T psum f32 ---
                    ytp = ps_y.tile([c96, PSB], F32, tag="ytp")
                    runs = s2_by_block.get(pb, [])
                    for ri, (a, w, k) in enumerate(runs):
                        nc.tensor.matmul(
                            out=ytp[:, a - pb * PSB : a - pb * PSB + w],
                            lhsT=wct[:, k, :],
                            rhs=gts[:, a - pb * PSB : a - pb * PSB + w],
                            start=(ri == 0),
                            stop=(ri == len(runs) - 1),
                        )
                    # zero exactly the uncovered column intervals (runs may
                    # leave mid-block gaps at 128-granular dw boundaries)
                    pos0 = 0
                    for a, w, _k in sorted(runs):
                        ra = a - pb * PSB
                        if ra > pos0:
                            nc.vector.memset(ytp[:, pos0:ra], 0.0)
                        pos0 = ra + w
                    if pos0 < PSB:
                        nc.vector.memset(ytp[:, pos0:PSB], 0.0)
                    yts = pyt.tile([c96, PSB], BF16, tag="yts")
                    nc.scalar.copy(out=yts[:], in_=ytp[:])
                    # --- transpose back -> Y psum bf16 [128, 4, 96] ---
                    ybp = ps_t.tile([128, 4, c96], BF16, tag="ybp")
                    for cch in range(4):
                        nc.tensor.transpose(
                            out=ybp[:, cch, :],
                            in_=yts[:, cch * 128 : (cch + 1) * 128],
                            identity=identb[0:c96, 0:c96],
                        )
                    ybs = pyb.tile([128, 4, c96], BF16, tag="ybs")
                    nc.scalar.copy(out=ybs[:], in_=ybp[:])
                    # --- SEG matmuls into acc^T psum ---
                    for cch in range(4):
                        ch = pb * 4 + cch
                        dwg = int(chunk_dw[ch])
                        if dwg < 0:
                            continue
                        if state["acc_ps_dw"] != dwg:
                            if state["acc_ps"] is not None:
                                retire_dw()
                            acc_new = ps_a.tile([c96, D], F32, tag="accps")
                            state["acc_ps"] = acc_new
                            state["acc_ps_dw"] = dwg
                            # open the accumulation with the center-offset
                            # contribution: acc^T += W_c^T @ x^T[:, window]
                            nc.tensor.matmul(
                                out=state["acc_ps"][:],
                                lhsT=wct[:, KC, :],
                                rhs=xlt[:, dwg * D : (dwg + 1) * D],
                                start=True, stop=False,
                            )
                        # build SEG [128, D] bf16 on DVE
                        segt = pseg.tile([128, D], BF16, tag="seg")
                        nc.vector.tensor_scalar(
                            out=segt[:],
                            in0=iota128[:],
                            scalar1=didc[:, blk2 * 4 + cch : blk2 * 4 + cch + 1],
                            scalar2=None,
                            op0=mybir.AluOpType.is_equal,
                        )
                        last = (ch + 1 == nchunk) or (int(chunk_dw[ch + 1]) != dwg)
                        nc.tensor.matmul(
                            out=state["acc_ps"][:],
                            lhsT=ybs[:, cch, :],
                            rhs=segt[:],
                            start=False,
                            stop=last,
                        )
            # retire last dw
            if state["acc_ps"] is not None:
                retire_dw()

    nc.compile()
    return nc


def make_inputs(xF, W_conv, ln_w, ln_b, W1, W2, gamma, nbr_idx, n_cores):
    plan, percore = build_plan(nbr_idx, n_cores)
    nc = build_nc(plan)
    npc, half, half_pad = plan["npc"], plan["half"], plan["half_pad"]
    wc = np.ascontiguousarray(W_conv.astype(ml_dtypes.bfloat16))
    lnvec = np.stack([ln_w, ln_b, gamma]).astype(np.float32)
    cpc = CALL // 128
    in_maps = []
    for c in range(n_cores):
        pc = percore[c]
        pool = build_pool(xF, pc["lo"][0], pc["lo"][1], plan["nblkh"])
        gidx = wrap_idx_calls(pc["gidx"], plan["ncall"])
        dest = pc["dest"].reshape(plan["ncall"], cpc, 128).transpose(0, 2, 1).copy()
        xloc = np.zeros((2 * half_pad, 96), dtype=np.float32)
        for h in range(2):
            i0 = c * npc + h * half
            n = half if h == 0 else npc - half
            xloc[h * half_pad : h * half_pad + n] = xF[i0 : i0 + n]
        in_maps.append({
            "pool": pool, "gidx": gidx, "dest": dest, "wc": wc,
            "iotad": np.arange(plan["D"], dtype=ml_dtypes.bfloat16).reshape(1, -1),
            "xloc": xloc,
            "xlt": np.ascontiguousarray(xloc.T.astype(ml_dtypes.bfloat16)),
            "w1": np.ascontiguousarray(W1, np.float32),
            "w2": np.ascontiguousarray(W2, np.float32),
            "lnvec": lnvec,
        })
    return nc, in_maps, plan


def kernel(xF, W_conv, ln_w, ln_b, W1, W2, gamma, nbr_idx, _profile=False):
    xF = np.asarray(xF, np.float32)
    nbr_idx = np.asarray(nbr_idx, np.int32)
    n_cores = 8
    nc, in_maps, plan = make_inputs(
        xF, np.asarray(W_conv, np.float32), np.asarray(ln_w, np.float32),
        np.asarray(ln_b, np.float32), np.asarray(W1, np.float32),
        np.asarray(W2, np.float32), np.asarray(gamma, np.float32),
        nbr_idx, n_cores,
    )
    res = run_bass_kernel_spmd(nc, in_maps, core_ids=list(range(n_cores)),
                               trace=_profile)
    npc, half, half_pad = plan["npc"], plan["half"], plan["half_pad"]
    outs = []
    for c in range(n_cores):
        o = res.results[c]["out"]
        outs.append(o[0:half])
        outs.append(o[half_pad : half_pad + (npc - half)])
    full = np.concatenate(outs, 0).astype(np.float32)
    if _profile:
        kernel.last_results = res
    return full
